# revision 6
# baseline (speedup 1.0000x reference)
"""Trainium2 Bass kernel for Conv2DCollapse_w_pillar (pillar scatter -> dense BEV).

One-hot matmul scatter, data-parallel over batch (1 batch / core):
  - Host: dedup pillar rows per flat cell (last write wins), sort by cell,
    bucket into 256-cell blocks (pairs of blocks share one matmul), pad to
    per-upload-group uniform row heights.  Features bf16 (rel err ~1.7e-3,
    gate is 2e-2).
  - Input upload coalesced into per-group DMAs split across the SP and ACT
    HWDGE queues (issue is 650ns/DMA per queue; 36 fine-grained DMAs used to
    gate the whole input phase).  cells table uploads as bf16 (integer cell
    ids are exact) and is upcast to f32 on the idle ACT engine.
  - Per pair of blocks: DVE/Pool build a one-hot oh[k, j] = (cell[k] == j)
    (tensor_scalar is_equal, 4x DVE mode); one bf16 matmul scatters the pair
    into PSUM f32 (128 partitions = 2 blocks x 64 channels); ACT/DVE drain
    PSUM to SBUF with the exact f32->bf16 cast (GPSIMD cannot read PSUM on
    TRN2, so Pool cannot drain); big 512B-run DMAs write the dense (C, ny*nx)
    bf16 plane.  Every output element is written exactly once.
  - GRP=4 pairs per PSUM tile with 4 pinned PSUM buffers keeps the
    matmul->drain ring 4 deep; drains spread ACT:DVE = 3:1; one-hots
    DVE:Pool ~= 11:5; expansions (packed chunk -> block-diagonal stationary
    layout) emitted EXLEAD steps early on DVE; 14 outb buffers absorb the
    drain/DMA rate mismatch with backlog built during the input phase.
"""
import sys
sys.path.insert(0, "/opt/trn_rl_repo")
import numpy as np
import ml_dtypes

BF = ml_dtypes.bfloat16
NCORES = 8
C = 64
NX = 512
NY = 512
NXY = NX * NY
BC = 256                 # cells per block
NBLK = NXY // BC         # 1024 blocks per core
NPAIR = NBLK // 2        # 512 pairs per core
CHUNK = 32               # pairs per feature-expansion chunk
NCHUNK = NPAIR // CHUNK
GRP = 4                  # pairs per PSUM group (2 banks)
P = 128                  # partition rows: even rows at 0:K, odd at 64:64+K
OG = 4                   # PSUM groups per outb DMA
# input DMA groups: chunk ranges with uniform row height per group
GROUPS = ((0, 1), (1, 2), (2, 4), (4, 8), (8, 12), (12, 16))

_cache = {}


def _build_nc(K, KE, KO, LEAD=8, EXLEAD=8, OUTB_BUFS=14,
              DRAIN_PAT="ADAAADAAADAAADAA", OH_PAT="DDPDDPDDPDDDPDDP",
              OH_RING=16, SWAP=0, PREWAIT=0):
    """KE/KO: per-GROUP padded row heights (len == len(GROUPS))."""
    import concourse.bass as bass
    import concourse.tile as tile
    from concourse import bacc, mybir
    from contextlib import ExitStack

    dt = mybir.dt
    assert len(KE) == len(KO) == len(GROUPS)
    assert max(max(KE), max(KO)) <= K <= 64
    KP = 64 + K
    nc = bacc.Bacc("TRN2", target_bir_lowering=False, debug=False,
                   num_devices=NCORES)
    fe_d, fo_d = [], []
    for gi, (lo, hi) in enumerate(GROUPS):
        span = hi - lo
        fe_d.append(nc.dram_tensor(f"fe{gi}", [KE[gi], span * CHUNK * C],
                                   dt.bfloat16, kind="ExternalInput").ap())
        fo_d.append(nc.dram_tensor(f"fo{gi}", [KO[gi], span * CHUNK * C],
                                   dt.bfloat16, kind="ExternalInput").ap())
    cells_d = nc.dram_tensor("cells", [P, NPAIR], dt.bfloat16,
                             kind="ExternalInput").ap()
    iota_d = nc.dram_tensor("iota", [P, BC], dt.bfloat16,
                            kind="ExternalInput").ap()
    zeros_d = None
    if K < 64:
        zeros_d = nc.dram_tensor("zeros", [64 - K, CHUNK * 128], dt.bfloat16,
                                 kind="ExternalInput").ap()
    # bf16 output: every value is exactly bf16-representable (features were
    # bf16-rounded; each output cell is a single such value or 0), so the
    # f32->bf16 drain cast and the host bf16->f32 upcast are both exact.
    out_d = nc.dram_tensor("out", [C, NXY], dt.bfloat16,
                           kind="ExternalOutput").ap()

    with tile.TileContext(nc) as tc, ExitStack() as ctx:
        const = ctx.enter_context(tc.tile_pool(name="const", bufs=1))
        featp = ctx.enter_context(tc.tile_pool(name="feat", bufs=1))
        lhsp = ctx.enter_context(tc.tile_pool(name="lhs", bufs=1))
        ohp = ctx.enter_context(tc.tile_pool(name="oh", bufs=1))
        outp = ctx.enter_context(tc.tile_pool(name="outb", bufs=OUTB_BUFS))
        psp = ctx.enter_context(tc.tile_pool(name="ps", bufs=1, space="PSUM"))

        iota_t = const.tile([P, BC], dt.bfloat16)
        cells_b = const.tile([P, NPAIR], dt.bfloat16)
        cells_t = const.tile([P, NPAIR], dt.float32)

        # packed features: partitions 0:K = even rows, 64:64+K = odd rows
        fb = featp.tile([P, NPAIR * C], dt.bfloat16, tag="fb", name="fb")
        fb3 = fb[:].rearrange("k (p f) -> k p f", f=C)

        # persistent stationary tiles (block-diagonal layout), double-buffered
        NLHS = 2
        lhs = [lhsp.tile([P, CHUNK * 128], dt.bfloat16,
                         tag=f"lhs{b}", name=f"lhs{b}") for b in range(NLHS)]
        z0 = lhs[0][:].rearrange("k (p f) -> k p f", f=128)
        z1 = lhs[1][:].rearrange("k (p f) -> k p f", f=128)

        # rows K:64 feed the PE (contraction is 0:KP) but are never written
        # by expansions: zero them via DMA (cheap).  The never-written column
        # halves of rows 0:K / 64:64+K get strided memsets (could hold NaN
        # bit patterns and 0*NaN = NaN): lhs0 halves on DVE (fast 4x, done
        # before cells arrive), lhs1 halves on ACT (idle until first drain).
        # Chunks 0/1 are forced to height K so the first expansion fully
        # initializes the live parts.
        if K < 64:
            nc.gpsimd.dma_start(lhs[0][K:64, :], zeros_d[:])
            nc.gpsimd.dma_start(lhs[1][K:64, :], zeros_d[:])
        nc.vector.memset(z0[0:K, :, C:128], 0.0)
        nc.vector.memset(z0[64:64 + K, :, 0:C], 0.0)
        nc.gpsimd.memset(z1[0:K, :, C:128], 0.0)
        nc.gpsimd.memset(z1[64:64 + K, :, 0:C], 0.0)

        # input upload: cells/iota first on SP (one-hot critical path), fe
        # groups on SP, fo groups on ACT.  All dep-free (fb persistent), so
        # no head-of-line risk on either queue.
        nc.sync.dma_start(cells_b[:], cells_d[:])
        nc.sync.dma_start(iota_t[:], iota_d[:])
        # bf16 holds the integer cell ids (and -1) exactly; upcast on ACT
        # (idle until the first drain) to the f32 the is_equal scalar needs
        nc.scalar.copy(cells_t[:], cells_b[:])
        for gi, (lo, hi) in enumerate(GROUPS):
            span = hi - lo
            nc.sync.dma_start(
                fb3[0:KE[gi], lo * CHUNK:hi * CHUNK, :],
                fe_d[gi][:].rearrange("k (p f) -> k p f", f=C))
            nc.scalar.dma_start(
                fb3[64:64 + KO[gi], lo * CHUNK:hi * CHUNK, :],
                fo_d[gi][:].rearrange("k (p f) -> k p f", f=C))

        # per-chunk heights (group height of the containing group)
        ke_c = [0] * NCHUNK
        ko_c = [0] * NCHUNK
        for gi, (lo, hi) in enumerate(GROUPS):
            for c in range(lo, hi):
                ke_c[c] = KE[gi]
                ko_c[c] = KO[gi]

        oh_tiles = {}

        def emit_oh(p):
            oh = ohp.tile([P, BC], dt.bfloat16, tag=f"oh{p % OH_RING}",
                          name=f"oh{p % OH_RING}")
            oh_eng = nc.gpsimd if OH_PAT[p % len(OH_PAT)] == "P" else nc.vector
            oh_eng.tensor_scalar(
                oh[0:KP, :], iota_t[0:KP, :], cells_t[0:KP, p:p + 1], None,
                mybir.AluOpType.is_equal)
            oh_tiles[p] = (oh, None)

        assert LEAD >= EXLEAD

        def emit_exp(c):
            # expand packed chunk into block-diagonal stationary layout
            # (DVE 4x copy mode, ~0.6us each); emitted EXLEAD steps before
            # the chunk's first matmul so it never sits between a blocked
            # drain and the one-hots the next matmul group needs
            buf = c % NLHS
            p0 = c * CHUNK
            t3 = lhs[buf][:].rearrange("k (p f) -> k p f", f=128)
            nc.vector.tensor_copy(
                t3[0:ke_c[c], :, 0:C],
                fb3[0:ke_c[c], p0:p0 + CHUNK, :])
            nc.vector.tensor_copy(
                t3[64:64 + ko_c[c], :, C:128],
                fb3[64:64 + ko_c[c], p0:p0 + CHUNK, :])

        NGRP = NPAIR // GRP
        PSBUFS = 4
        # Matmul-group emission order: a group whose PSUM buffer is freed by
        # a slow Pool drain (g = pool_g + PSBUFS) is swapped with its
        # successor, whose buffer was freed by a fast ACT drain.  PSUM tags
        # stay pinned to the ORIGINAL group index, so the swapped-early
        # successor proceeds immediately and the stalled group gets one
        # extra group of calendar slack before it must dispatch.
        order = list(range(NGRP))
        if SWAP:
            for g in range(PSBUFS, NGRP - 1):
                if (DRAIN_PAT[(g - PSBUFS) % len(DRAIN_PAT)] == "P"
                        and order[g] == g and order[g + 1] == g + 1):
                    order[g], order[g + 1] = order[g + 1], order[g]
        qseq = [g * GRP + i for g in order for i in range(GRP)]

        gq = {}    # group -> psum tile
        ob = {}    # outb index -> sbuf tile
        done = {}  # outb index -> drained group count
        for step in range(NPAIR + LEAD):
            if step < NPAIR:
                emit_oh(step)
            s = step - LEAD
            qe = s + EXLEAD
            if qe >= 0 and qe % CHUNK == 0 and qe // CHUNK < NCHUNK:
                emit_exp(qe // CHUNK)
            if s < 0:
                continue
            q = qseq[s]
            g, i = divmod(q, GRP)
            buf = (q // CHUNK) % NLHS
            ok = g // OG
            if i == 0:
                if ok not in ob:
                    ob[ok] = outp.tile([128, OG * GRP * BC], dt.bfloat16,
                                       name="outb")
                gq[g] = psp.tile([128, GRP * BC], dt.float32,
                                 tag=f"ps{g % PSBUFS}", name=f"ps{g % PSBUFS}")
            ps_t = gq[g]
            sl = q % CHUNK
            oh, j = oh_tiles.pop(q)
            rhs = (oh[0:KP, :] if j is None else
                   oh[0:KP, :].rearrange("k (c j) -> k j c", j=4)[:, j, :])
            nc.tensor.matmul(
                ps_t[:, i * BC:(i + 1) * BC],
                lhs[buf][0:KP, sl * 128:(sl + 1) * 128],
                rhs,
                start=True, stop=True)
            if i == GRP - 1:
                outb = ob[ok]
                half = (g % OG) * GRP * BC
                # drain PSUM -> SBUF with exact f32->bf16 cast
                de = DRAIN_PAT[g % len(DRAIN_PAT)]
                if de == "D":
                    nc.vector.tensor_copy(outb[:, half:half + GRP * BC], ps_t[:])
                elif de == "P":
                    nc.gpsimd.tensor_copy(outb[:, half:half + GRP * BC], ps_t[:])
                else:
                    nc.scalar.copy(outb[:, half:half + GRP * BC], ps_t[:])
                del gq[g]
                done[ok] = done.get(ok, 0) + 1
                if done[ok] == OG:
                    base = ok * OG * GRP * 2 * BC
                    dst4 = out_d[:, base:base + OG * GRP * 2 * BC].rearrange(
                        "c (p q r) -> c p q r", p=OG * GRP, q=2, r=BC)
                    src_e = outb[0:C, :].rearrange("c (p r) -> c p r", r=BC)
                    src_o = outb[C:128, :].rearrange("c (p r) -> c p r", r=BC)
                    nc.sync.dma_start(dst4[:, :, 0, :], src_e)
                    nc.sync.dma_start(dst4[:, :, 1, :], src_o)
                    del ob[ok], done[ok]
    nc.compile()
    return nc


def _prep_core(pf, cell, K, KE, KO):
    """pf: (Nb, C) f32 features for this batch (deduped, sorted by cell);
    cell: (Nb,) int cell ids.  KE/KO: per-group padded row heights."""
    n = len(cell)
    block = cell // BC
    local = (cell % BC).astype(np.float32)
    starts = np.searchsorted(block, np.arange(NBLK))
    k = np.arange(n) - starts[block]
    assert k.max(initial=0) < K
    pair = block // 2
    parity = block % 2

    feat = pf.astype(BF)

    ev = parity == 0
    od = ~ev
    fe = np.zeros((K, NPAIR, C), dtype=BF)
    fo = np.zeros((K, NPAIR, C), dtype=BF)
    fe[k[ev], pair[ev], :] = feat[ev]
    fo[k[od], pair[od], :] = feat[od]
    m = {}
    for gi, (lo, hi) in enumerate(GROUPS):
        span = hi - lo
        m[f"fe{gi}"] = np.ascontiguousarray(
            fe[:KE[gi], lo * CHUNK:hi * CHUNK, :].reshape(KE[gi],
                                                          span * CHUNK * C))
        m[f"fo{gi}"] = np.ascontiguousarray(
            fo[:KO[gi], lo * CHUNK:hi * CHUNK, :].reshape(KO[gi],
                                                          span * CHUNK * C))
    cells = np.full((P, NPAIR), -1.0, np.float32)
    cells[k[ev], pair[ev]] = local[ev]
    cells[64 + k[od], pair[od]] = local[od]
    m["cells"] = cells.astype(BF)
    m["iota"] = np.broadcast_to(
        np.arange(BC, dtype=np.float32), (P, BC)).astype(BF).copy()
    if K < 64:
        m["zeros"] = np.zeros((64 - K, CHUNK * 128), dtype=BF)
    if K < 64:
        m["zeros"] = np.zeros((64 - K, CHUNK * 128), dtype=BF)
    return m


def _plan(cell_by_core):
    """Compute K and per-group heights from per-core cell id arrays."""
    nchunk = NCHUNK
    KEa = np.zeros(nchunk, np.int64)
    KOa = np.zeros(nchunk, np.int64)
    for cells in cell_by_core:
        occ = np.bincount(cells // BC, minlength=NBLK)
        KEa = np.maximum(KEa, occ[0::2].reshape(nchunk, CHUNK).max(axis=1))
        KOa = np.maximum(KOa, occ[1::2].reshape(nchunk, CHUNK).max(axis=1))
    K = int(max(KEa.max(), KOa.max(), 8))
    KE, KO = [], []
    for gi, (lo, hi) in enumerate(GROUPS):
        ke = int(KEa[lo:hi].max())
        ko = int(KOa[lo:hi].max())
        # chunks 0 and 1 (the first use of each lhs buffer) must fully
        # initialize rows 0:K / 64:64+K
        if lo <= 0 < hi or lo <= 1 < hi:
            ke = ko = K
        KE.append(ke)
        KO.append(ko)
    return K, tuple(KE), tuple(KO)


def kernel(pillar_features, coords, batch_size, nx, ny, num_bev_features,
           **_ignored):
    from concourse import bass_utils

    pf = np.ascontiguousarray(np.asarray(pillar_features, dtype=np.float32))
    co = np.asarray(coords).astype(np.int64)
    B = int(batch_size)
    nx_i, ny_i, C_i = int(nx), int(ny), int(num_bev_features)
    assert (B, nx_i, ny_i, C_i) == (NCORES, NX, NY, C), "hardcoded shape mismatch"

    key = co[:, 0] * NXY + co[:, 1] + co[:, 2] * NX + co[:, 3]
    # dedup, last occurrence wins (matches reference .at[].set semantics)
    n = len(key)
    u, first_rev = np.unique(key[::-1], return_index=True)
    src = n - 1 - first_rev           # original row index that survives
    batch = (u // NXY).astype(np.int64)
    cell = (u % NXY).astype(np.int64)
    bstart = np.searchsorted(batch, np.arange(NCORES + 1))

    cell_by_core = [cell[bstart[b]:bstart[b + 1]] for b in range(NCORES)]
    K, KE, KO = _plan(cell_by_core)

    import os as _os
    _knobs = {}
    for _k in ("LEAD", "EXLEAD", "OUTB_BUFS", "OH_RING", "SWAP", "PREWAIT"):
        if _os.environ.get(f"KN_{_k}"):
            _knobs[_k] = int(_os.environ[f"KN_{_k}"])
    for _k in ("DRAIN_PAT", "OH_PAT"):
        if _os.environ.get(f"KN_{_k}"):
            _knobs[_k] = _os.environ[f"KN_{_k}"]
    _key = (K, KE, KO) if not _knobs \
        else (K, KE, KO, tuple(sorted(_knobs.items())))
    if _key not in _cache:
        _cache[_key] = _build_nc(K, KE, KO, **_knobs)
    nc = _cache[_key]

    in_maps = []
    for b in range(NCORES):
        lo_i, hi_i = bstart[b], bstart[b + 1]
        in_maps.append(_prep_core(pf[src[lo_i:hi_i]], cell_by_core[b],
                                  K, KE, KO))

    import os
    trace = bool(os.environ.get("BASS_TRACE"))
    res = bass_utils.run_bass_kernel_spmd(
        nc, in_maps, core_ids=list(range(NCORES)), trace=trace)
    kernel._last_results = res

    out = np.empty((NCORES, C, NY, NX), dtype=np.float32)
    for b in range(NCORES):
        out[b] = np.asarray(res.results[b]["out"]).astype(
            np.float32).reshape(C, NY, NX)
    return out


# revision 7
# speedup vs baseline: 1.0271x; 1.0271x over previous
"""Trainium2 Bass kernel for Conv2DCollapse_w_pillar (pillar scatter -> dense BEV).

One-hot matmul scatter, data-parallel over batch (1 batch / core):
  - Host: dedup pillar rows per flat cell (last write wins), sort by cell,
    bucket into 256-cell blocks (pairs of blocks share one matmul), pad to
    per-upload-group uniform row heights.  Features bf16 (rel err ~1.7e-3,
    gate is 2e-2).
  - Input upload coalesced into per-group DMAs split across the SP and ACT
    HWDGE queues (issue is 650ns/DMA per queue; 36 fine-grained DMAs used to
    gate the whole input phase).  cells table uploads as bf16 (integer cell
    ids are exact) and is upcast to f32 on the idle ACT engine.
  - Per pair of blocks: DVE/Pool build a one-hot oh[k, j] = (cell[k] == j)
    (tensor_scalar is_equal, 4x DVE mode); one bf16 matmul scatters the pair
    into PSUM f32 (128 partitions = 2 blocks x 64 channels); ACT/DVE drain
    PSUM to SBUF with the exact f32->bf16 cast (GPSIMD cannot read PSUM on
    TRN2, so Pool cannot drain); big 512B-run DMAs write the dense (C, ny*nx)
    bf16 plane.  Every output element is written exactly once.
  - GRP=4 pairs per PSUM tile with 4 pinned PSUM buffers keeps the
    matmul->drain ring 4 deep; drains spread ACT:DVE = 3:1; one-hots
    DVE:Pool ~= 11:5; expansions (packed chunk -> block-diagonal stationary
    layout) emitted EXLEAD steps early on DVE; 14 outb buffers absorb the
    drain/DMA rate mismatch with backlog built during the input phase.
"""
import sys
sys.path.insert(0, "/opt/trn_rl_repo")
import numpy as np
import ml_dtypes

BF = ml_dtypes.bfloat16
NCORES = 8
C = 64
NX = 512
NY = 512
NXY = NX * NY
BC = 256                 # cells per block
NBLK = NXY // BC         # 1024 blocks per core
NPAIR = NBLK // 2        # 512 pairs per core
CHUNK = 32               # pairs per feature-expansion chunk
NCHUNK = NPAIR // CHUNK
GRP = 4                  # pairs per PSUM group (2 banks)
P = 128                  # partition rows: even rows at 0:K, odd at 64:64+K
OG = 2                   # PSUM groups per outb DMA
# input DMA groups: chunk ranges with uniform row height per group
GROUPS = ((0, 1), (1, 2), (2, 4), (4, 8), (8, 12), (12, 16))

_cache = {}


def _build_nc(K, KE, KO, LEAD=12, EXLEAD=28, OUTB_BUFS=28,
              DRAIN_PAT="ADAAADAAADAAADAA", OH_PAT="DDPDDPDDPDDDPDDP",
              OH_RING=16, SWAP=0, PREWAIT=0, PREFIX_A=4):
    """KE/KO: per-GROUP padded row heights (len == len(GROUPS))."""
    import concourse.bass as bass
    import concourse.tile as tile
    from concourse import bacc, mybir
    from contextlib import ExitStack

    dt = mybir.dt
    assert len(KE) == len(KO) == len(GROUPS)
    assert max(max(KE), max(KO)) <= K <= 64
    KP = 64 + K
    nc = bacc.Bacc("TRN2", target_bir_lowering=False, debug=False,
                   num_devices=NCORES)
    fe_d, fo_d = [], []
    for gi, (lo, hi) in enumerate(GROUPS):
        span = hi - lo
        fe_d.append(nc.dram_tensor(f"fe{gi}", [KE[gi], span * CHUNK * C],
                                   dt.bfloat16, kind="ExternalInput").ap())
        fo_d.append(nc.dram_tensor(f"fo{gi}", [KO[gi], span * CHUNK * C],
                                   dt.bfloat16, kind="ExternalInput").ap())
    cells_d = nc.dram_tensor("cells", [P, NPAIR], dt.bfloat16,
                             kind="ExternalInput").ap()
    iota_d = nc.dram_tensor("iota", [P, BC], dt.bfloat16,
                            kind="ExternalInput").ap()
    zeros_d = None
    if K < 64:
        zeros_d = nc.dram_tensor("zeros", [64 - K, CHUNK * 128], dt.bfloat16,
                                 kind="ExternalInput").ap()
    # bf16 output: every value is exactly bf16-representable (features were
    # bf16-rounded; each output cell is a single such value or 0), so the
    # f32->bf16 drain cast and the host bf16->f32 upcast are both exact.
    out_d = nc.dram_tensor("out", [C, NXY], dt.bfloat16,
                           kind="ExternalOutput").ap()

    with tile.TileContext(nc) as tc, ExitStack() as ctx:
        const = ctx.enter_context(tc.tile_pool(name="const", bufs=1))
        featp = ctx.enter_context(tc.tile_pool(name="feat", bufs=1))
        lhsp = ctx.enter_context(tc.tile_pool(name="lhs", bufs=1))
        ohp = ctx.enter_context(tc.tile_pool(name="oh", bufs=1))
        outp = ctx.enter_context(tc.tile_pool(name="outb", bufs=OUTB_BUFS))
        psp = ctx.enter_context(tc.tile_pool(name="ps", bufs=1, space="PSUM"))

        iota_t = const.tile([P, BC], dt.bfloat16)
        cells_b = const.tile([P, NPAIR], dt.bfloat16)
        cells_t = const.tile([P, NPAIR], dt.float32)

        # packed features: partitions 0:K = even rows, 64:64+K = odd rows
        fb = featp.tile([P, NPAIR * C], dt.bfloat16, tag="fb", name="fb")
        fb3 = fb[:].rearrange("k (p f) -> k p f", f=C)

        # persistent stationary tiles (block-diagonal layout), double-buffered
        NLHS = 2
        lhs = [lhsp.tile([P, CHUNK * 128], dt.bfloat16,
                         tag=f"lhs{b}", name=f"lhs{b}") for b in range(NLHS)]
        z0 = lhs[0][:].rearrange("k (p f) -> k p f", f=128)
        z1 = lhs[1][:].rearrange("k (p f) -> k p f", f=128)

        # rows K:64 feed the PE (contraction is 0:KP) but are never written
        # by expansions: zero them via DMA (cheap).  The never-written column
        # halves of rows 0:K / 64:64+K get strided memsets (could hold NaN
        # bit patterns and 0*NaN = NaN): lhs0 halves on DVE (fast 4x, done
        # before cells arrive), lhs1 halves on ACT (idle until first drain).
        # Chunks 0/1 are forced to height K so the first expansion fully
        # initializes the live parts.
        if K < 64:
            nc.gpsimd.dma_start(lhs[0][K:64, :], zeros_d[:])
            nc.gpsimd.dma_start(lhs[1][K:64, :], zeros_d[:])
        nc.vector.memset(z0[0:K, :, C:128], 0.0)
        nc.vector.memset(z0[64:64 + K, :, 0:C], 0.0)
        nc.gpsimd.memset(z1[0:K, :, C:128], 0.0)
        nc.gpsimd.memset(z1[64:64 + K, :, 0:C], 0.0)

        # input upload: cells/iota first on SP (one-hot critical path), fe
        # groups on SP, fo groups on ACT.  All dep-free (fb persistent), so
        # no head-of-line risk on either queue.
        nc.sync.dma_start(cells_b[:], cells_d[:])
        nc.sync.dma_start(iota_t[:], iota_d[:])
        # bf16 holds the integer cell ids (and -1) exactly; upcast on ACT
        # (idle until the first drain) to the f32 the is_equal scalar needs
        nc.scalar.copy(cells_t[:], cells_b[:])
        for gi, (lo, hi) in enumerate(GROUPS):
            span = hi - lo
            nc.sync.dma_start(
                fb3[0:KE[gi], lo * CHUNK:hi * CHUNK, :],
                fe_d[gi][:].rearrange("k (p f) -> k p f", f=C))
            nc.scalar.dma_start(
                fb3[64:64 + KO[gi], lo * CHUNK:hi * CHUNK, :],
                fo_d[gi][:].rearrange("k (p f) -> k p f", f=C))

        # per-chunk heights (group height of the containing group)
        ke_c = [0] * NCHUNK
        ko_c = [0] * NCHUNK
        for gi, (lo, hi) in enumerate(GROUPS):
            for c in range(lo, hi):
                ke_c[c] = KE[gi]
                ko_c[c] = KO[gi]

        oh_tiles = {}

        def emit_oh(p):
            oh = ohp.tile([P, BC], dt.bfloat16, tag=f"oh{p % OH_RING}",
                          name=f"oh{p % OH_RING}")
            oh_eng = nc.gpsimd if OH_PAT[p % len(OH_PAT)] == "P" else nc.vector
            oh_eng.tensor_scalar(
                oh[0:KP, :], iota_t[0:KP, :], cells_t[0:KP, p:p + 1], None,
                mybir.AluOpType.is_equal)
            oh_tiles[p] = (oh, None)


        def emit_exp(c):
            # expand packed chunk into block-diagonal stationary layout
            # (DVE 4x copy mode, ~0.6us each); emitted EXLEAD steps before
            # the chunk's first matmul so it never sits between a blocked
            # drain and the one-hots the next matmul group needs
            buf = c % NLHS
            p0 = c * CHUNK
            t3 = lhs[buf][:].rearrange("k (p f) -> k p f", f=128)
            nc.vector.tensor_copy(
                t3[0:ke_c[c], :, 0:C],
                fb3[0:ke_c[c], p0:p0 + CHUNK, :])
            nc.vector.tensor_copy(
                t3[64:64 + ko_c[c], :, C:128],
                fb3[64:64 + ko_c[c], p0:p0 + CHUNK, :])

        NGRP = NPAIR // GRP
        PSBUFS = 4
        # Matmul-group emission order: a group whose PSUM buffer is freed by
        # a slow Pool drain (g = pool_g + PSBUFS) is swapped with its
        # successor, whose buffer was freed by a fast ACT drain.  PSUM tags
        # stay pinned to the ORIGINAL group index, so the swapped-early
        # successor proceeds immediately and the stalled group gets one
        # extra group of calendar slack before it must dispatch.
        order = list(range(NGRP))
        if SWAP:
            for g in range(PSBUFS, NGRP - 1):
                if (DRAIN_PAT[(g - PSBUFS) % len(DRAIN_PAT)] == "P"
                        and order[g] == g and order[g + 1] == g + 1):
                    order[g], order[g + 1] = order[g + 1], order[g]
        qseq = [g * GRP + i for g in order for i in range(GRP)]

        gq = {}    # group -> psum tile
        ob = {}    # outb index -> sbuf tile
        done = {}  # outb index -> drained group count
        for step in range(-max(0, EXLEAD - LEAD), NPAIR + LEAD):
            if 0 <= step < NPAIR:
                emit_oh(step)
            s = step - LEAD
            qe = s + EXLEAD
            if qe >= 0 and qe % CHUNK == 0 and qe // CHUNK < NCHUNK:
                emit_exp(qe // CHUNK)
            if s < 0:
                continue
            q = qseq[s]
            g, i = divmod(q, GRP)
            buf = (q // CHUNK) % NLHS
            ok = g // OG
            if i == 0:
                if ok not in ob:
                    ob[ok] = outp.tile([128, OG * GRP * BC], dt.bfloat16,
                                       name="outb")
                gq[g] = psp.tile([128, GRP * BC], dt.float32,
                                 tag=f"ps{g % PSBUFS}", name=f"ps{g % PSBUFS}")
            ps_t = gq[g]
            sl = q % CHUNK
            oh, j = oh_tiles.pop(q)
            rhs = (oh[0:KP, :] if j is None else
                   oh[0:KP, :].rearrange("k (c j) -> k j c", j=4)[:, j, :])
            nc.tensor.matmul(
                ps_t[:, i * BC:(i + 1) * BC],
                lhs[buf][0:KP, sl * 128:(sl + 1) * 128],
                rhs,
                start=True, stop=True)
            if i == GRP - 1:
                outb = ob[ok]
                half = (g % OG) * GRP * BC
                # drain PSUM -> SBUF with exact f32->bf16 cast
                de = "A" if g < PREFIX_A else DRAIN_PAT[g % len(DRAIN_PAT)]
                if de == "D":
                    nc.vector.tensor_copy(outb[:, half:half + GRP * BC], ps_t[:])
                elif de == "P":
                    nc.gpsimd.tensor_copy(outb[:, half:half + GRP * BC], ps_t[:])
                else:
                    nc.scalar.copy(outb[:, half:half + GRP * BC], ps_t[:])
                del gq[g]
                done[ok] = done.get(ok, 0) + 1
                if done[ok] == OG:
                    base = ok * OG * GRP * 2 * BC
                    dst4 = out_d[:, base:base + OG * GRP * 2 * BC].rearrange(
                        "c (p q r) -> c p q r", p=OG * GRP, q=2, r=BC)
                    src_e = outb[0:C, :].rearrange("c (p r) -> c p r", r=BC)
                    src_o = outb[C:128, :].rearrange("c (p r) -> c p r", r=BC)
                    nc.sync.dma_start(dst4[:, :, 0, :], src_e)
                    nc.sync.dma_start(dst4[:, :, 1, :], src_o)
                    del ob[ok], done[ok]
    nc.compile()
    return nc


def _prep_core(pf, cell, K, KE, KO):
    """pf: (Nb, C) f32 features for this batch (deduped, sorted by cell);
    cell: (Nb,) int cell ids.  KE/KO: per-group padded row heights."""
    n = len(cell)
    block = cell // BC
    local = (cell % BC).astype(np.float32)
    starts = np.searchsorted(block, np.arange(NBLK))
    k = np.arange(n) - starts[block]
    assert k.max(initial=0) < K
    pair = block // 2
    parity = block % 2

    feat = pf.astype(BF)

    ev = parity == 0
    od = ~ev
    fe = np.zeros((K, NPAIR, C), dtype=BF)
    fo = np.zeros((K, NPAIR, C), dtype=BF)
    fe[k[ev], pair[ev], :] = feat[ev]
    fo[k[od], pair[od], :] = feat[od]
    m = {}
    for gi, (lo, hi) in enumerate(GROUPS):
        span = hi - lo
        m[f"fe{gi}"] = np.ascontiguousarray(
            fe[:KE[gi], lo * CHUNK:hi * CHUNK, :].reshape(KE[gi],
                                                          span * CHUNK * C))
        m[f"fo{gi}"] = np.ascontiguousarray(
            fo[:KO[gi], lo * CHUNK:hi * CHUNK, :].reshape(KO[gi],
                                                          span * CHUNK * C))
    cells = np.full((P, NPAIR), -1.0, np.float32)
    cells[k[ev], pair[ev]] = local[ev]
    cells[64 + k[od], pair[od]] = local[od]
    m["cells"] = cells.astype(BF)
    m["iota"] = np.broadcast_to(
        np.arange(BC, dtype=np.float32), (P, BC)).astype(BF).copy()
    if K < 64:
        m["zeros"] = np.zeros((64 - K, CHUNK * 128), dtype=BF)
    if K < 64:
        m["zeros"] = np.zeros((64 - K, CHUNK * 128), dtype=BF)
    return m


def _plan(cell_by_core):
    """Compute K and per-group heights from per-core cell id arrays."""
    nchunk = NCHUNK
    KEa = np.zeros(nchunk, np.int64)
    KOa = np.zeros(nchunk, np.int64)
    for cells in cell_by_core:
        occ = np.bincount(cells // BC, minlength=NBLK)
        KEa = np.maximum(KEa, occ[0::2].reshape(nchunk, CHUNK).max(axis=1))
        KOa = np.maximum(KOa, occ[1::2].reshape(nchunk, CHUNK).max(axis=1))
    K = int(max(KEa.max(), KOa.max(), 8))
    KE, KO = [], []
    for gi, (lo, hi) in enumerate(GROUPS):
        ke = int(KEa[lo:hi].max())
        ko = int(KOa[lo:hi].max())
        # chunks 0 and 1 (the first use of each lhs buffer) must fully
        # initialize rows 0:K / 64:64+K
        if lo <= 0 < hi or lo <= 1 < hi:
            ke = ko = K
        KE.append(ke)
        KO.append(ko)
    return K, tuple(KE), tuple(KO)


def kernel(pillar_features, coords, batch_size, nx, ny, num_bev_features,
           **_ignored):
    from concourse import bass_utils

    pf = np.ascontiguousarray(np.asarray(pillar_features, dtype=np.float32))
    co = np.asarray(coords).astype(np.int64)
    B = int(batch_size)
    nx_i, ny_i, C_i = int(nx), int(ny), int(num_bev_features)
    assert (B, nx_i, ny_i, C_i) == (NCORES, NX, NY, C), "hardcoded shape mismatch"

    key = co[:, 0] * NXY + co[:, 1] + co[:, 2] * NX + co[:, 3]
    # dedup, last occurrence wins (matches reference .at[].set semantics)
    n = len(key)
    u, first_rev = np.unique(key[::-1], return_index=True)
    src = n - 1 - first_rev           # original row index that survives
    batch = (u // NXY).astype(np.int64)
    cell = (u % NXY).astype(np.int64)
    bstart = np.searchsorted(batch, np.arange(NCORES + 1))

    cell_by_core = [cell[bstart[b]:bstart[b + 1]] for b in range(NCORES)]
    K, KE, KO = _plan(cell_by_core)

    import os as _os
    _knobs = {}
    for _k in ("LEAD", "EXLEAD", "OUTB_BUFS", "OH_RING", "SWAP", "PREWAIT", "PREFIX_A"):
        if _os.environ.get(f"KN_{_k}"):
            _knobs[_k] = int(_os.environ[f"KN_{_k}"])
    for _k in ("DRAIN_PAT", "OH_PAT"):
        if _os.environ.get(f"KN_{_k}"):
            _knobs[_k] = _os.environ[f"KN_{_k}"]
    _key = (K, KE, KO) if not _knobs \
        else (K, KE, KO, tuple(sorted(_knobs.items())))
    if _key not in _cache:
        _cache[_key] = _build_nc(K, KE, KO, **_knobs)
    nc = _cache[_key]

    in_maps = []
    for b in range(NCORES):
        lo_i, hi_i = bstart[b], bstart[b + 1]
        in_maps.append(_prep_core(pf[src[lo_i:hi_i]], cell_by_core[b],
                                  K, KE, KO))

    import os
    trace = bool(os.environ.get("BASS_TRACE"))
    res = bass_utils.run_bass_kernel_spmd(
        nc, in_maps, core_ids=list(range(NCORES)), trace=trace)
    kernel._last_results = res

    out = np.empty((NCORES, C, NY, NX), dtype=np.float32)
    for b in range(NCORES):
        out[b] = np.asarray(res.results[b]["out"]).astype(
            np.float32).reshape(C, NY, NX)
    return out
